# revision 35
# baseline (speedup 1.0000x reference)
"""DSTMamba Trainium2 kernel: 8 NeuronCores, SPMD.

Core c handles (batch b=c//2, direction d=c%2). Odd cores receive the
token axis (n) reversed so the same forward-scan program computes the
reverse-direction Mamba branch; the bidirectional merge is a pair
AllReduce (bf16) + subtract-own-contribution + reversed add.

Engine plan (v2):
- PE: all matmuls (bf16 weights) + identity-matmul PSUM accumulation
  for the sum-over-states reduction of the selective scan.
- ACT: dA = exp(-(s+1)*dt) generation (bf16), fused psum evacuations
  (conv scale+bias, silu, gelu, softplus via exp/ln pinned to set 6).
- DVE: the 64 tensor_tensor_scan ops per layer (only engine that can
  scan) + bf16 2x mul share.
- Pool (gpsimd): tensor_tensor mul/add share (SBUF-only, same-dtype),
  freed from DMA descriptor generation.
- SP (sync): all DMAs via HWDGE queue (weights, broadcasts, exchange).
- Collectives in bf16; trend-branch compute emitted inside the two
  collective windows.
- B/C rows broadcast via one DRAM round-trip, streamed to SBUF in
  4-state quarters; scan runs s-outer over g-pairs so quarters retire.
"""

import numpy as np
import ml_dtypes

import concourse.bacc as bacc
import concourse.mybir as mybir
from concourse import tile
from concourse.bass_utils import run_bass_kernel_spmd

B, L, H, N = 4, 512, 96, 862
DM, DS = 256, 16
DI = 512
DTR = 16
DFF, NLAYERS = 256, 2
DSL, KSTD = 3, 25
EPS = 1e-5

F32 = mybir.dt.float32
F32R = mybir.dt.float32r
BF16 = mybir.dt.bfloat16
AL = mybir.AluOpType
AF = mybir.ActivationFunctionType

NC2 = [(0, 512), (512, 350)]  # psum-bank-sized moving-dim chunks of N=862
PAIRS = [[0, 1], [2, 3], [4, 5], [6, 7]]
QS = 4  # B/C broadcast streaming chunk: states per chunk

_CACHE = {}
import os
DEBUG = int(os.environ.get("KDBG", "0"))


def _dbg(c, name, aps):
    if not DEBUG:
        return
    nc = c.nc
    rows = sum(a.shape[0] for a in aps)
    o = nc.dram_tensor(f"dbg_{name}", [rows, N], mybir.dt.float32,
                       kind="ExternalOutput").ap()
    r0 = 0
    for a in aps:
        r = a.shape[0]
        nc.gpsimd.dma_start(o[r0:r0 + r, :], a)
        r0 += r


# ---------------------------------------------------------------- host math
def _mavg_matrix(length):
    M = np.zeros((length, length), np.float64)
    p = (KSTD - 1) // 2
    for i in range(length):
        for d in range(-p, p + 1):
            j = min(max(i + d, 0), length - 1)
            M[i, j] += 1.0 / KSTD
    return M


def _pool_matrix(lo, hi):
    P = np.zeros((lo, hi), np.float64)
    for i in range(lo):
        P[i, 2 * i] = 0.5
        P[i, 2 * i + 1] = 0.5
    return P


def _trend_ops():
    ops = []
    P = np.eye(L)
    cur = L
    for s in range(DSL + 1):
        ops.append(_mavg_matrix(cur) @ P)
        if s < DSL:
            P = _pool_matrix(cur // 2, cur) @ P
            cur //= 2
    return ops  # [512,512],[256,512],[128,512],[64,512]


def _col(v):
    v = np.asarray(v, np.float32).reshape(-1)
    if v.size <= 128:
        return np.ascontiguousarray(v.reshape(-1, 1))
    return np.ascontiguousarray(v.reshape(-1, 128).T)


def _row(v):
    return np.ascontiguousarray(np.asarray(v, np.float32).reshape(1, -1))


def _t(m):
    return np.ascontiguousarray(np.asarray(m, np.float32).T)


def _tb(m):
    return np.ascontiguousarray(
        np.asarray(m, np.float32).T.astype(ml_dtypes.bfloat16))


def make_core_inputs(inputs, core):
    b, d = core // 2, core % 2
    g = lambda k: np.asarray(inputs[k], np.float32)

    m = {}
    x = g("history_data")[b, :, :, 0]
    if d == 1:
        x = x[:, ::-1]
    m["x_in"] = np.ascontiguousarray(x)

    tops = _trend_ops()
    for s in range(4):
        m[f"trop{s}_T"] = _tb(tops[s])

    # fused inverted-embedding: xt = emb_w @ (I - M0) @ xn
    m["emb_lhsT"] = _tb(g("emb_w") @ (np.eye(L) - tops[0]))
    m["emb_b"] = _col(g("emb_b"))
    m["ident"] = np.eye(128, dtype=ml_dtypes.bfloat16)

    for l in range(NLAYERS):
        m[f"in_lhsT_{l}"] = _tb(g("m_in")[l, d])
        m[f"cw0_{l}"] = _col(g("m_conv_w")[l, d, :, 0])
        m[f"cw1_{l}"] = _col(g("m_conv_w")[l, d, :, 1])
        m[f"cb_{l}"] = _col(g("m_conv_b")[l, d])
        m[f"dtb_{l}"] = _col(g("m_dt_b")[l, d])
        m[f"D_{l}"] = _col(g("m_D")[l, d])
        m[f"n1w_{l}"] = _col(g("n1_w")[l])
        m[f"n1b_{l}"] = _col(g("n1_b")[l])
        m[f"n2w_{l}"] = _col(g("n2_w")[l])
        m[f"n2b_{l}"] = _col(g("n2_b")[l])
        m[f"f1b_{l}"] = _col(g("f1_b")[l])
        m[f"f2b_{l}"] = _col(g("f2_b")[l])
        xpt = _t(g("m_xproj")[l, d]).astype(ml_dtypes.bfloat16)
        m[f"xpbc_lhsT_{l}"] = np.ascontiguousarray(xpt[:, DTR:])
        m[f"xpdt_lhsT_{l}"] = np.ascontiguousarray(xpt[:, :DTR])
        m[f"dt_lhsT_{l}"] = _tb(g("m_dt_w")[l, d])
        m[f"out_lhsT_{l}"] = _tb(g("m_out")[l, d])
        m[f"f1_lhsT_{l}"] = _tb(g("f1_w")[l])
        m[f"f2_lhsT_{l}"] = _tb(g("f2_w")[l])

    m["encnw"] = _col(g("encn_w"))
    m["encnb"] = _col(g("encn_b"))
    m["proj_lhsT"] = _tb(g("proj_w"))
    m["projb"] = _col(g("proj_b"))

    for i in range(DSL):
        m[f"u{i}w1_lhsT"] = _tb(g(f"u{i}w1"))
        m[f"u{i}b1"] = _col(g(f"u{i}b1"))
        m[f"u{i}w2_lhsT"] = _tb(g(f"u{i}w2"))
        m[f"u{i}b2"] = _col(g(f"u{i}b2"))
    for s in range(4):
        m[f"map{s}_lhsT"] = _tb(g(f"map{s}_w"))
    m["mapb"] = _col(sum(g(f"map{s}_b") for s in range(4)))

    rvw, rvb, trw = g("revin_w"), g("revin_b"), g("tre_w")
    if d == 1:
        rvw, rvb, trw = rvw[::-1], rvb[::-1], trw[::-1]
    m["rvw_row"] = _row(rvw)
    m["rvb_row"] = _row(rvb)
    m["trw_row"] = _row(trw)
    m["ones_col"] = np.ones((128, 1), np.float32)
    m["ones_row"] = np.ones((1, 128), np.float32)
    m["ones_bf"] = np.ones((128, 1), ml_dtypes.bfloat16)
    return m


# ---------------------------------------------------- act-table pinning
class _Bacc(bacc.Bacc):
    """Resolve Exp/Ln to act-set 6 (which holds both) instead of
    flip-flopping between sets 0 and 5: present the insertion pass a
    table list with exp/ln masked out of the sets we don't want.
    Indices stay canonical, so the emitted act_func_set_id is valid."""

    def insert_act_table_loads(self):
        import bass_rust as _bass_rust
        from concourse.hw_specs import get_activation_tables

        has_activation = any(
            isinstance(i, mybir.InstActivation)
            for b in self.main_func.blocks
            for i in b.instructions
        )
        if not has_activation:
            return
        tables = []
        for name, fns in get_activation_tables(self.m.arch).items():
            fns = set(fns)
            if name in ("exp_and_others", "exp_and_friends"):
                fns.discard(AF.Exp)
            if name == "natural_log":
                fns.discard(AF.Ln)
            tables.append((name, fns))
        _bass_rust.insert_act_table_loads(self, tables)


# ------------------------------------------------------------- device build
class _Ctx:
    pass


def _build():
    nc = _Bacc("TRN2", target_bir_lowering=False, debug=False,
               num_devices=8)

    def din(name, shape, dt=F32):
        return nc.dram_tensor(name, list(shape), dt, kind="ExternalInput").ap()

    I = {}
    I["x_in"] = din("x_in", [L, N], F32R)
    for s, ls in enumerate([512, 256, 128, 64]):
        I[f"trop{s}_T"] = din(f"trop{s}_T", [L, ls], BF16)
    I["emb_lhsT"] = din("emb_lhsT", [L, DM], BF16)
    I["emb_b"] = din("emb_b", [128, DM // 128])
    I["ident"] = din("ident", [128, 128], BF16)
    for l in range(NLAYERS):
        I[f"in_lhsT_{l}"] = din(f"in_lhsT_{l}", [DM, 2 * DI], BF16)
        for k in ["cw0", "cw1", "cb", "dtb", "D"]:
            I[f"{k}_{l}"] = din(f"{k}_{l}", [128, DI // 128])
        I[f"xpbc_lhsT_{l}"] = din(f"xpbc_lhsT_{l}", [DI, 2 * DS], BF16)
        I[f"xpdt_lhsT_{l}"] = din(f"xpdt_lhsT_{l}", [DI, DTR], BF16)
        I[f"dt_lhsT_{l}"] = din(f"dt_lhsT_{l}", [DTR, DI], BF16)
        I[f"out_lhsT_{l}"] = din(f"out_lhsT_{l}", [DI, DM], BF16)
        for k in ["n1w", "n1b", "n2w", "n2b", "f1b", "f2b"]:
            I[f"{k}_{l}"] = din(f"{k}_{l}", [128, DM // 128])
        I[f"f1_lhsT_{l}"] = din(f"f1_lhsT_{l}", [DM, DFF], BF16)
        I[f"f2_lhsT_{l}"] = din(f"f2_lhsT_{l}", [DFF, DM], BF16)
    I["encnw"] = din("encnw", [128, DM // 128])
    I["encnb"] = din("encnb", [128, DM // 128])
    I["proj_lhsT"] = din("proj_lhsT", [DM, H], BF16)
    I["projb"] = din("projb", [H, 1])
    for i, (li, lo) in enumerate([(64, 128), (128, 256), (256, 512)]):
        I[f"u{i}w1_lhsT"] = din(f"u{i}w1_lhsT", [li, lo], BF16)
        I[f"u{i}b1"] = din(f"u{i}b1", [min(lo, 128), max(1, lo // 128)])
        I[f"u{i}w2_lhsT"] = din(f"u{i}w2_lhsT", [lo, lo], BF16)
        I[f"u{i}b2"] = din(f"u{i}b2", [min(lo, 128), max(1, lo // 128)])
    for s, ls in enumerate([512, 256, 128, 64]):
        I[f"map{s}_lhsT"] = din(f"map{s}_lhsT", [ls, H], BF16)
    I["mapb"] = din("mapb", [H, 1])
    for k in ["rvw_row", "rvb_row", "trw_row"]:
        I[k] = din(k, [1, N])
    I["ones_col"] = din("ones_col", [128, 1], F32R)
    I["ones_row"] = din("ones_row", [1, 128], F32R)
    I["ones_bf"] = din("ones_bf", [128, 1], BF16)

    out_pred = nc.dram_tensor("pred", [H, N], F32, kind="ExternalOutput").ap()

    c = _Ctx()
    c.nc, c.I, c.out_pred = nc, I, out_pred
    with tile.TileContext(nc) as tc:
        c.tc = tc
        _emit(c)
    nc.compile()
    return nc


def _load(c, pool, key, tag=None, eng=None):
    ap = c.I[key]
    t_ = pool.tile(list(ap.shape), ap.dtype, name=key, tag=tag or key)
    (eng or c.nc.gpsimd).dma_start(t_[:, :], ap[:, :])
    return t_


def _load_tiles(c, pool, key, tag=None, eng=None, split=False):
    """One DMA per weight tensor; returns [ko][mo] 128x<=128 views."""
    ap = c.I[key]
    K, M = ap.shape
    eng = eng or c.nc.gpsimd
    if split:
        out = []
        for ko in range(0, K, 128):
            rowt = []
            for mo in range(0, M, 128):
                kk, mm = min(128, K - ko), min(128, M - mo)
                t_ = pool.tile([kk, mm], ap.dtype, name=f"{key}_{ko}_{mo}",
                               tag=f"{tag or key}_{ko}_{mo}")
                eng.dma_start(t_[:, :], ap[ko:ko + kk, mo:mo + mm])
                rowt.append(t_)
            out.append(rowt)
        return out
    if K <= 128:
        t_ = pool.tile([K, M], ap.dtype, name=key, tag=tag or key)
        eng.dma_start(t_[:, :], ap[:, :])
        return [[t_[:, mo:mo + min(128, M - mo)] for mo in range(0, M, 128)]]
    KT = K // 128
    assert K % 128 == 0, (key, K)
    t_ = pool.tile([128, KT * M], ap.dtype, name=key, tag=tag or key)
    eng.dma_start(t_[:, :].rearrange("p (k m) -> p k m", k=KT),
                  ap[:, :].rearrange("(k p) m -> p k m", k=KT))
    out = []
    for ko in range(KT):
        rowt = []
        for mo in range(0, M, 128):
            mm = min(128, M - mo)
            rowt.append(t_[:, ko * M + mo:ko * M + mo + mm])
        out.append(rowt)
    return out


def _pe_bcast(c, psum_pool, row_ap, parts, nparts_tag, dt_=F32R):
    """Broadcast a [1,N] sbuf row into per-chunk PSUM tiles via PE
    (ones-row outer product). Returns {n0: psum_tile}."""
    nc = c.nc
    out = {}
    for n0, nl in NC2:
        ps = psum_pool.tile([parts, nl], F32, name=f"bc{nparts_tag}{n0}",
                            tag=f"{nparts_tag}{n0}")
        nc.tensor.matmul(ps[:, :], c.ones_row[:1, :parts],
                         row_ap[:1, n0:n0 + nl],
                         start=True, stop=True)
        out[n0] = ps
    return out


def _bcast(c, pool, row_ap, parts, tag, dt=F32, via_dram=True):
    """broadcast [1,N] (sbuf or dram) row to [parts, N] sbuf tile."""
    nc = c.nc
    if via_dram:
        d = c.dp.tile([1, N], dt, name=f"bd_{tag}", tag=f"bd_{tag}")
        nc.gpsimd.dma_start(d[:, :], row_ap)
        src = d[:, :]
    else:
        src = row_ap
    bt = pool.tile([parts, N], dt, name=f"bc_{tag}", tag=f"bc_{tag}")
    nc.gpsimd.dma_start(bt[:, :], src.broadcast_to([parts, N]))
    return bt


def _matsum(c, psum, lhs_tiles, rhs_tiles, n0, nl):
    """psum += sum_k lhs_tiles[k].T @ rhs_tiles[k][:, n0:n0+nl]"""
    nc = c.nc
    kn = len(lhs_tiles)
    for k in range(kn):
        nc.tensor.matmul(psum[:, :], lhs_tiles[k][:, :],
                         rhs_tiles[k][:, n0:n0 + nl],
                         start=(k == 0), stop=(k == kn - 1))


def _layer_norm(c, scr, xin, wcol, bcol, outpool, outtag, pool_mi=(0,)):
    """xin: 2 [128,N] bf16 tiles -> 2 [128,N] bf16 tiles (norm over 256).

    Stats via ones-matmul on PE; mean/rstd rows broadcast back through
    PE into PSUM (no DRAM round-trip); pointwise on DVE.
    """
    nc, pm, tc = c.nc, c.pm, c.tc
    mrow = scr.tile([1, N], F32R, name=f"lnm_{outtag}", tag="ln_mrow", bufs=1)
    qrow = scr.tile([1, N], F32, name=f"lnq_{outtag}", tag="ln_qrow", bufs=1)
    m2row = scr.tile([1, N], F32, name=f"ln2_{outtag}", tag="ln_m2row", bufs=1)
    rrow = scr.tile([1, N], F32R, name=f"lnr_{outtag}", tag="ln_rrow", bufs=1)
    sq = []
    for mi in range(2):
        s_ = scr.tile([128, N], F32R, name=f"lnsq{mi}", tag=f"ln_sq{mi}",
                      bufs=1)
        nc.scalar.activation(s_[:, :], xin[mi][:, :], AF.Square)
        sq.append(s_)
    for n0, nl in NC2:
        ps = pm.tile([1, nl], F32, name="lnps", tag="mm1")
        for mi in range(2):
            nc.tensor.matmul(ps[:, :], c.ones_bf[:, :], xin[mi][:, n0:n0 + nl],
                             start=(mi == 0), stop=(mi == 1))
        nc.vector.tensor_scalar_mul(mrow[:, n0:n0 + nl], ps[:, :], 1.0 / DM)
    nc.vector.tensor_tensor(m2row[:, :], mrow[:, :].bitcast(F32),
                            mrow[:, :].bitcast(F32), AL.mult)
    for n0, nl in NC2:
        ps2 = pm.tile([1, nl], F32, name="lnps2", tag="mm1")
        for mi in range(2):
            nc.tensor.matmul(ps2[:, :], c.ones_col[:, :], sq[mi][:, n0:n0 + nl],
                             start=(mi == 0), stop=(mi == 1))
        # qrow = ps2/DM - m2row
        nc.vector.scalar_tensor_tensor(qrow[:, n0:n0 + nl], ps2[:, :],
                                       1.0 / DM, m2row[:, n0:n0 + nl],
                                       AL.mult, AL.subtract)
    nc.scalar.activation(qrow[:, :], qrow[:, :], AF.Ln, bias=c.epscol[:1, :])
    nc.scalar.activation(rrow[:, :], qrow[:, :], AF.Exp, scale=-0.5)
    out = []
    with tc.tile_pool(name=f"lnb_{outtag}", bufs=1, space="PSUM") as pb:
        mb = _pe_bcast(c, pb, mrow, 128, "lnb")
        d1 = []
        for mi in range(2):
            d_ = scr.tile([128, N], BF16, name=f"lnd1_{mi}",
                          tag=f"ln_d1{mi}", bufs=1)
            for n0, nl in NC2:
                nc.vector.tensor_tensor(d_[:, n0:n0 + nl], xin[mi][:, n0:n0 + nl],
                                        mb[n0][:, :], AL.subtract)
            d1.append(d_)
        rb = _pe_bcast(c, pb, rrow, 128, "lnb")
        for mi in range(2):
            o = outpool.tile([128, N], BF16, name=f"{outtag}{mi}",
                             tag=f"{outtag}{mi}")
            for n0, nl in NC2:
                nc.vector.tensor_tensor(d1[mi][:, n0:n0 + nl],
                                        d1[mi][:, n0:n0 + nl],
                                        rb[n0][:, :], AL.mult)
            nc.vector.tensor_scalar(o[:, :], d1[mi][:, :],
                                    wcol[:, mi:mi + 1],
                                    bcol[:, mi:mi + 1], AL.mult, AL.add)
            out.append(o)
    return out


def _emit(c):
    nc, tc, I = c.nc, c.tc, c.I
    import contextlib
    with contextlib.ExitStack() as est:
        gp = est.enter_context(tc.tile_pool(name="glob", bufs=1))
        pm = est.enter_context(tc.tile_pool(name="pmm", bufs=2, space="PSUM"))
        dp = est.enter_context(tc.tile_pool(name="drm", bufs=1, space="DRAM"))
        c.gp, c.pm, c.dp = gp, pm, dp

        c.ones_col = _load(c, gp, "ones_col")
        c.ones_row = _load(c, gp, "ones_row")
        c.ones_bf = _load(c, gp, "ones_bf")
        epscol = gp.tile([128, 1], F32, name="epscol", tag="epscol")
        c.nc.gpsimd.memset(epscol[:, :], EPS)
        c.epscol = epscol
        c.ident = _load(c, gp, "ident")
        r_mean = gp.tile([1, N], F32, name="r_mean", tag="r_mean")
        r_sc = gp.tile([1, N], F32R, name="r_sc", tag="r_sc")
        c.r_mean, c.r_sc = r_mean, r_sc

        # ======================================================== stage A+B
        with tc.tile_pool(name="front", bufs=1) as fp:
            r_msq = fp.tile([1, N], F32, name="r_msq", tag="r_msq")
            r_std = fp.tile([1, N], F32, name="r_std", tag="r_std")
            r_wr = fp.tile([1, N], F32R, name="r_wr", tag="r_wr")
            Xall = fp.tile([128, 4 * N], F32R, name="xinall", tag="xinall")
            nc.gpsimd.dma_start(
                Xall[:, :].rearrange("p (c n) -> p c n", c=4),
                I["x_in"][:, :].rearrange("(c p) n -> p c n", c=4))
            X = [Xall[:, ci * N:(ci + 1) * N] for ci in range(4)]
            for n0, nl in NC2:
                ps = pm.tile([1, nl], F32, name="rvs", tag="mm1")
                for ci in range(4):
                    nc.tensor.matmul(ps[:, :], c.ones_col[:, :],
                                     X[ci][:, n0:n0 + nl],
                                     start=(ci == 0), stop=(ci == 3))
                nc.scalar.activation(r_mean[:, n0:n0 + nl], ps[:, :],
                                     AF.Copy, scale=1.0 / L)
            sqt = []
            for ci in range(4):
                sq = fp.tile([128, N], F32R, name=f"rvsq{ci}", tag=f"sq{ci}")
                nc.scalar.activation(sq[:, :], X[ci][:, :].bitcast(F32),
                                     AF.Square)
                sqt.append(sq)
            m2 = fp.tile([1, N], F32, name="rvm2", tag="rvm2")
            nc.scalar.activation(m2[:, :], r_mean[:, :], AF.Square)
            for n0, nl in NC2:
                ps2 = pm.tile([1, nl], F32, name="rvq", tag="mm1")
                for ci in range(4):
                    nc.tensor.matmul(ps2[:, :], c.ones_col[:, :],
                                     sqt[ci][:, n0:n0 + nl],
                                     start=(ci == 0), stop=(ci == 3))
                nc.vector.scalar_tensor_tensor(r_msq[:, n0:n0 + nl], ps2[:, :],
                                               1.0 / L, m2[:, n0:n0 + nl],
                                               AL.mult, AL.subtract)
            nc.scalar.activation(r_msq[:, :], r_msq[:, :], AF.Ln,
                                 bias=c.epscol[:1, :])
            nc.scalar.activation(r_std[:, :], r_msq[:, :], AF.Exp, scale=0.5)
            nc.scalar.activation(r_wr[:, :], r_msq[:, :], AF.Exp, scale=-0.5)
            rvw = fp.tile([1, N], F32, name="rvwrow", tag="rvwrow")
            nc.gpsimd.dma_start(rvw[:, :], I["rvw_row"][:, :])
            nc.vector.tensor_tensor(r_wr[:, :], r_wr[:, :].bitcast(F32),
                                    rvw[:, :], AL.mult)
            # sc = std / (rvw + 1e-10)   (for final denorm)
            t1 = fp.tile([1, N], F32, name="sct1", tag="sct1")
            nc.vector.tensor_scalar_add(t1[:, :], rvw[:, :], 1e-10)
            nc.vector.reciprocal(t1[:, :], t1[:, :])
            nc.vector.tensor_tensor(r_sc[:, :], t1[:, :], r_std[:, :], AL.mult)

            # xn = X*(rvw/std) - (mean*(rvw/std) - rvb), via PE broadcasts
            rvbr = fp.tile([1, N], F32, name="rvbr", tag="rvbr")
            nc.gpsimd.dma_start(rvbr[:, :], I["rvb_row"][:, :])
            s2 = fp.tile([1, N], F32R, name="s2row", tag="s2row")
            nc.vector.tensor_tensor(s2[:, :], r_mean[:, :],
                                    r_wr[:, :].bitcast(F32), AL.mult)
            nc.vector.tensor_tensor(s2[:, :], s2[:, :].bitcast(F32),
                                    rvbr[:, :], AL.subtract)
            c.xn = []
            with tc.tile_pool(name="fbc", bufs=1, space="PSUM") as pb:
                wbP = _pe_bcast(c, pb, r_wr, 128, "fw")
                s2P = _pe_bcast(c, pb, s2, 128, "fs")
                for ci in range(4):
                    o = gp.tile([128, N], BF16, name=f"xn{ci}", tag=f"xn{ci}")
                    d1 = fp.tile([128, N], BF16, name=f"rvd{ci}",
                                 tag=f"rvd{ci % 2}")
                    for n0, nl in NC2:
                        nc.vector.tensor_tensor(d1[:, n0:n0 + nl],
                                                X[ci][:, n0:n0 + nl].bitcast(F32),
                                                wbP[n0][:, :], AL.mult)
                        nc.vector.tensor_tensor(o[:, n0:n0 + nl],
                                                d1[:, n0:n0 + nl],
                                                s2P[n0][:, :], AL.subtract)
                    c.xn.append(o)

            _dbg(c, "xn", [t[:, :] for t in c.xn])
            EL = _load_tiles(c, fp, "emb_lhsT", split=True)
            if DEBUG:
                ow = c.nc.dram_tensor("dbg_elw", [128, 8 * 128],
                                      mybir.dt.float32,
                                      kind="ExternalOutput").ap()
                idx = 0
                for ko in range(4):
                    for mo in range(2):
                        c.nc.gpsimd.dma_start(
                            ow[:, idx * 128:(idx + 1) * 128], EL[ko][mo])
                        idx += 1
            emb_b = _load(c, fp, "emb_b")
            xt = []
            for mc in range(2):
                t_ = gp.tile([128, N], BF16, name=f"xtA{mc}", tag=f"xtA{mc}")
                xt.append(t_)
                for n0, nl in NC2:
                    ps = pm.tile([128, nl], F32, name="embmm", tag="mm")
                    _matsum(c, ps, [EL[k][mc] for k in range(4)], c.xn, n0, nl)
                    nc.scalar.activation(t_[:, n0:n0 + nl], ps[:, :],
                                         AF.Identity,
                                         bias=emb_b[:, mc:mc + 1])

        _dbg(c, "xt", [t[:, :] for t in xt])
        # ======================================================== encoder
        for l in range(NLAYERS):
            with contextlib.ExitStack() as lst:
                lp = lst.enter_context(tc.tile_pool(name=f"lay{l}", bufs=1))
                rp = lst.enter_context(tc.tile_pool(name=f"rot{l}", bufs=2))
                xt = _mamba_layer(c, l, lp, rp, xt)

        # ======================================================== tail
        with contextlib.ExitStack() as tst:
            tp = tst.enter_context(tc.tile_pool(name="tail", bufs=1))
            encw = _load(c, tp, "encnw")
            encb = _load(c, tp, "encnb")
            xf = _layer_norm(c, tp, xt, encw, encb, c.gp, "xtB")
            PRJ = _load_tiles(c, tp, "proj_lhsT")
            projb = _load(c, tp, "projb")
            seaT = tp.tile([H, N], F32, name="seaT", tag="seaT")
            for n0, nl in NC2:
                ps = pm.tile([H, nl], F32, name="prmm", tag="mm")
                _matsum(c, ps, [PRJ[k][0] for k in range(2)], xf, n0, nl)
                nc.scalar.activation(seaT[:, n0:n0 + nl], ps[:, :], AF.Identity,
                                     bias=projb[:, :])

            treT = c.treT

            # final combine + RevIN denorm:
            # pred = seaT*sc + treT*(trw*sc) + (mean - rvb*sc)
            r2 = tp.tile([1, N], F32R, name="finr2", tag="finr2")
            trw = tp.tile([1, N], F32, name="trwrow", tag="trwrow")
            nc.gpsimd.dma_start(trw[:, :], I["trw_row"][:, :])
            nc.vector.tensor_tensor(r2[:, :], trw[:, :],
                                    c.r_sc[:, :].bitcast(F32), AL.mult)
            r3 = tp.tile([1, N], F32R, name="finr3", tag="finr3")
            rvb = tp.tile([1, N], F32, name="rvbrow", tag="rvbrow")
            nc.gpsimd.dma_start(rvb[:, :], I["rvb_row"][:, :])
            nc.vector.tensor_tensor(r3[:, :], rvb[:, :],
                                    c.r_sc[:, :].bitcast(F32), AL.mult)
            nc.vector.tensor_tensor(r3[:, :], c.r_mean[:, :],
                                    r3[:, :].bitcast(F32), AL.subtract)
            p1 = tp.tile([H, N], F32, name="fin1", tag="fin1")
            p2 = tp.tile([H, N], F32, name="fin2", tag="fin2")
            with tc.tile_pool(name="tbc", bufs=1, space="PSUM") as pb:
                scP = _pe_bcast(c, pb, c.r_sc, H, "ta")
                r2P = _pe_bcast(c, pb, r2, H, "tb")
                for n0, nl in NC2:
                    nc.vector.tensor_tensor(p1[:, n0:n0 + nl],
                                            seaT[:, n0:n0 + nl],
                                            scP[n0][:, :], AL.mult)
                    nc.vector.tensor_tensor(p2[:, n0:n0 + nl],
                                            treT[:, n0:n0 + nl],
                                            r2P[n0][:, :], AL.mult)
                r3P = _pe_bcast(c, pb, r3, H, "ta")
                for n0, nl in NC2:
                    nc.vector.tensor_tensor(p1[:, n0:n0 + nl], p1[:, n0:n0 + nl],
                                            p2[:, n0:n0 + nl], AL.add)
                    nc.vector.tensor_tensor(p1[:, n0:n0 + nl], p1[:, n0:n0 + nl],
                                            r3P[n0][:, :], AL.add)
            nc.gpsimd.dma_start(c.out_pred[:, :], p1[:, :])


def _trend_extract(c, tp):
    """trend matmuls from xn (independent of encoder) -> c.trt"""
    nc, pm = c.nc, c.pm
    trt = []
    for s, ls in enumerate([512, 256, 128, 64]):
        with c.tc.tile_pool(name=f"wtr{s}", bufs=1) as wtr:
            TR = _load_tiles(c, wtr, f"trop{s}_T", eng=nc.gpsimd)
            mt = []
            for mc in range((ls + 127) // 128):
                parts = min(128, ls - mc * 128)
                t_ = tp.tile([parts, N], BF16, name=f"tr{s}_{mc}",
                             tag=f"tr{s}_{mc}")
                mt.append(t_)
                for n0, nl in NC2:
                    ps = pm.tile([parts, nl], F32, name="trmm", tag="mm")
                    _matsum(c, ps, [TR[k][mc] for k in range(4)], c.xn,
                            n0, nl)
                    nc.scalar.copy(t_[:, n0:n0 + nl], ps[:, :])
            trt.append(mt)
    c.trt = trt


def _trend_mix(c, tp):
    """TimeMixer trend mixing + maps -> c.treT (uses c.trt)."""
    nc, pm = c.nc, c.pm
    tr0, tr1, tr2, tr3 = c.trt

    def mixstep(low, i, high):
        with c.tc.tile_pool(name=f"wu{i}", bufs=1) as wu:
            W1 = _load_tiles(c, wu, f"u{i}w1_lhsT", eng=nc.gpsimd)
            W2 = _load_tiles(c, wu, f"u{i}w2_lhsT", eng=nc.gpsimd)
            b1 = _load(c, wu, f"u{i}b1", eng=nc.gpsimd)
            b2 = _load(c, wu, f"u{i}b2", eng=nc.gpsimd)
            gt = []
            for mc in range(len(W1[0])):
                parts = W1[0][mc].shape[1]
                g_ = tp.tile([parts, N], BF16, name=f"mxg{i}_{mc}",
                             tag=f"gA{mc}")
                gt.append(g_)
                for n0, nl in NC2:
                    ps = pm.tile([parts, nl], F32, name="mxmm", tag="mm")
                    _matsum(c, ps, [W1[k][mc] for k in range(len(W1))],
                            low, n0, nl)
                    nc.scalar.activation(
                        g_[:, n0:n0 + nl], ps[:, :], AF.Gelu,
                        bias=b1[:parts, mc:mc + 1])
            out = []
            for mc in range(len(W2[0])):
                parts = W2[0][mc].shape[1]
                o_ = high[mc]  # accumulate in place into the trend tile
                out.append(o_)
                b_ = tp.tile([parts, N], BF16, name=f"mxb{i}_{mc}", tag="mxb",
                             bufs=2)
                for n0, nl in NC2:
                    ps = pm.tile([parts, nl], F32, name="mxmm2", tag="mm")
                    _matsum(c, ps, [W2[k][mc] for k in range(len(W2))],
                            gt, n0, nl)
                    nc.scalar.activation(
                        b_[:, n0:n0 + nl], ps[:, :], AF.Identity,
                        bias=b2[:parts, mc:mc + 1])
                nc.gpsimd.tensor_tensor(o_[:, :], o_[:, :], b_[:, :], AL.add)
            return out

    o1 = mixstep(tr3, 0, tr2)
    o2 = mixstep(o1, 1, tr1)
    o3 = mixstep(o2, 2, tr0)

    with c.tc.tile_pool(name="wmap", bufs=1) as wm:
        MP = [_load_tiles(c, wm, f"map{s}_lhsT", eng=nc.gpsimd)
              for s in range(4)]
        mapb = _load(c, wm, "mapb", eng=nc.gpsimd)
        outst = [o3, o2, o1, tr3]
        treT = tp.tile([H, N], F32, name="treT", tag="treT")
        for n0, nl in NC2:
            ps = c.pm.tile([H, nl], F32, name="mpmm", tag="mm")
            ops = []
            for s in range(4):
                for k in range(len(MP[s])):
                    ops.append((MP[s][k][0], outst[s][k]))
            for i, (w_, x_) in enumerate(ops):
                nc.tensor.matmul(ps[:, :], w_[:, :], x_[:, n0:n0 + nl],
                                 start=(i == 0), stop=(i == len(ops) - 1))
            nc.scalar.activation(treT[:, n0:n0 + nl], ps[:, :], AF.Identity,
                                 bias=mapb[:, :])
    c.treT = treT


def _mamba_layer(c, l, lp, rp, xt):
    nc, pm = c.nc, c.pm
    tc = c.tc

    # ---------------- in_proj + conv + silu + z-gate (act set 18 region)
    cw0 = _load(c, lp, f"cw0_{l}")
    cw1 = _load(c, lp, f"cw1_{l}")
    cb = _load(c, lp, f"cb_{l}")
    xcs, zr = [], []
    IL = _load_tiles(c, lp, f"in_lhsT_{l}")
    with tc.tile_pool(name=f"pin{l}", bufs=2, space="PSUM") as pin:
        for g in range(4):
            # xc part: psum per chunk; evac fused with conv scale+bias
            xcc = rp.tile([128, N], BF16, name=f"xcc{g}", tag="xcc", bufs=2)
            pss = []
            for n0, nl in NC2:
                ps = pin.tile([128, nl], F32, name=f"inps{g}",
                              tag=f"inps{n0}")
                pss.append(ps)
                _matsum(c, ps, [IL[k][g] for k in range(2)], xt, n0, nl)
                nc.scalar.activation(xcc[:, n0:n0 + nl], ps[:, :], AF.Identity,
                                     scale=cw1[:, g:g + 1],
                                     bias=cb[:, g:g + 1])
            # shifted tap: xcc[:,1:] += xcraw[:, :N-1]*cw0 (psum operand)
            n1 = NC2[0][1]
            nc.vector.scalar_tensor_tensor(
                xcc[:, 1:n1], pss[0][:, 0:n1 - 1],
                cw0[:, g:g + 1], xcc[:, 1:n1], AL.mult, AL.add)
            nc.vector.scalar_tensor_tensor(
                xcc[:, n1:n1 + 1], pss[0][:, n1 - 1:n1],
                cw0[:, g:g + 1], xcc[:, n1:n1 + 1], AL.mult, AL.add)
            nc.vector.scalar_tensor_tensor(
                xcc[:, n1 + 1:], pss[1][:, 0:N - n1 - 1],
                cw0[:, g:g + 1], xcc[:, n1 + 1:], AL.mult, AL.add)
            o = lp.tile([128, N], BF16, name=f"xcs{g}", tag=f"xcs{g}")
            nc.scalar.activation(o[:, :], xcc[:, :], AF.Silu)
            xcs.append(o)
    # ---------------- x_proj -> B,C rows; dt (softplus via exp/ln, set 6)
    dtT = []
    bc_dram = c.dp.tile([1, 32 * N], BF16, name=f"bcd{l}", tag="bc_dram")
    with tc.tile_pool(name=f"w2_{l}", bufs=1) as wp2:
        XPB = _load_tiles(c, wp2, f"xpbc_lhsT_{l}")
        XPD = _load_tiles(c, wp2, f"xpdt_lhsT_{l}")
        dtin = lp.tile([16, N], BF16, name="dtin", tag="dtin")
        bcrows = lp.tile([32, N], BF16, name="bcrows", tag="bcrows")
        for n0, nl in NC2:
            ps = pm.tile([32, nl], F32, name="xpmm", tag="mm")
            _matsum(c, ps, [XPB[k][0] for k in range(4)], xcs, n0, nl)
            nc.scalar.copy(bcrows[:, n0:n0 + nl], ps[:, :])
            ps2 = pm.tile([16, nl], F32, name="xpmm2", tag="mm")
            _matsum(c, ps2, [XPD[k][0] for k in range(4)], xcs, n0, nl)
            nc.scalar.copy(dtin[:, n0:n0 + nl], ps2[:, :])
        nc.gpsimd.dma_start(bc_dram[:, :].rearrange("a (r n) -> (a r) n", r=32),
                            bcrows[:, :])
        DTW = _load_tiles(c, wp2, f"dt_lhsT_{l}")
        dtb = _load(c, lp, f"dtb_{l}")
        for g in range(4):
            u = rp.tile([128, N], BF16, name=f"dtu{g}", tag="dtu", bufs=2)
            for n0, nl in NC2:
                ps = pm.tile([128, nl], F32, name="dtmm", tag="mm")
                nc.tensor.matmul(ps[:, :], DTW[0][g][:, :], dtin[:, n0:n0 + nl],
                                 start=True, stop=True)
                nc.scalar.activation(u[:, n0:n0 + nl], ps[:, :], AF.Exp,
                                     bias=dtb[:, g:g + 1])
            dt_ = lp.tile([128, N], BF16, name=f"dtT{g}", tag=f"dtT{g}")
            nc.scalar.activation(dt_[:, :], u[:, :], AF.Ln, bias=1.0)
            dtT.append(dt_)
    if l == 0:
        _dbg(c, "xcs", [t[:, :] for t in xcs])
        _dbg(c, "dtT", [t[:, :] for t in dtT])
        _dbg(c, "bcr", [bcrows[:, :]])
    wT = []
    for g in range(4):
        w_ = lp.tile([128, N], BF16, name=f"wT{g}", tag=f"wT{g}")
        nc.vector.tensor_tensor(w_[:, :], dtT[g][:, :], xcs[g][:, :], AL.mult)
        wT.append(w_)
    def _emit_z(g):
        zt = lp.tile([128, N], BF16, name=f"zr{g}", tag=f"zr{g}")
        zr.append(zt)
        for n0, nl in NC2:
            ps = pm.tile([128, nl], F32, name="inmm", tag="mm")
            _matsum(c, ps, [IL[k][4 + g] for k in range(2)], xt, n0, nl)
            nc.scalar.activation(zt[:, n0:n0 + nl], ps[:, :], AF.Silu)

    for g in range(2):
        _emit_z(g)

    # ---------------- selective scan: s-outer over g-pairs, quarters of
    # B/C broadcast streamed from DRAM, PE identity-matmul psum reduction
    Dcol = _load(c, lp, f"D_{l}")
    ym = []
    ctr = 0
    with tc.tile_pool(name=f"pda{l}", bufs=1, space="PSUM") as pa:
        ypsum = {}
        for gi in range(2):
            for n0, nl in NC2:
                ypsum[(gi, n0)] = pa.tile([128, nl], F32, name=f"y{gi}_{n0}",
                                          tag=f"y{gi}_{n0}", bufs=1)
        for half in range(2):
            gs = (0, 1) if half == 0 else (2, 3)
            if half == 1:
                for g in (2, 3):
                    _emit_z(g)
            for q in range(16 // QS):
                qB = lp.tile([128, QS * N], BF16, name="qB", tag="qB", bufs=2)
                nc.gpsimd.dma_start(
                    qB[:, :],
                    bc_dram[:1, q * QS * N:(q + 1) * QS * N]
                    .broadcast_to([128, QS * N]))
                qC = lp.tile([128, QS * N], BF16, name="qC", tag="qC", bufs=2)
                nc.gpsimd.dma_start(
                    qC[:, :],
                    bc_dram[:1, (16 + q * QS) * N:(16 + (q + 1) * QS) * N]
                    .broadcast_to([128, QS * N]))
                for si in range(QS):
                    s = q * QS + si
                    for gi, g in enumerate(gs):
                        da = rp.tile([128, N], BF16, name="da", tag="da",
                                     bufs=4)
                        nc.scalar.activation(da[:, :], dtT[g][:, :], AF.Exp,
                                             scale=float(-(s + 1)))
                        dbx = rp.tile([128, N], BF16, name="dbx", tag="dbx",
                                      bufs=4)
                        eng = nc.gpsimd if ctr % 4 == 0 else nc.vector
                        eng.tensor_tensor(dbx[:, :], wT[g][:, :],
                                          qB[:, si * N:(si + 1) * N], AL.mult)
                        h = rp.tile([128, N], BF16, name="h", tag="h", bufs=3)
                        nc.vector.tensor_tensor_scan(h[:, :], da[:, :],
                                                     dbx[:, :],
                                                     0.0, AL.mult, AL.add)
                        tmp = rp.tile([128, N], BF16, name="tmp", tag="tmp",
                                      bufs=3)
                        eng2 = nc.gpsimd if ctr % 3 == 0 else nc.vector
                        eng2.tensor_tensor(tmp[:, :], h[:, :],
                                           qC[:, si * N:(si + 1) * N], AL.mult)
                        for n0, nl in NC2:
                            nc.tensor.matmul(ypsum[(gi, n0)][:, :],
                                             c.ident[:, :],
                                             tmp[:, n0:n0 + nl],
                                             start=(s == 0), stop=(s == 15))
                        ctr += 1
            # gating ym = (y + D*xcs) * silu(z) for this g-pair
            for gi, g in enumerate(gs):
                yg = rp.tile([128, N], BF16, name=f"yg{g}", tag="yg", bufs=2)
                for n0, nl in NC2:
                    nc.vector.scalar_tensor_tensor(yg[:, n0:n0 + nl],
                                                   xcs[g][:, n0:n0 + nl],
                                                   Dcol[:, g:g + 1],
                                                   ypsum[(gi, n0)][:, :],
                                                   AL.mult, AL.add)
                o = lp.tile([128, N], BF16, name=f"ym{g}", tag=f"ym{g}")
                nc.vector.tensor_tensor(o[:, :], yg[:, :], zr[g][:, :],
                                        AL.mult)
                ym.append(o)

    if l == 0:
        _dbg(c, "ym", [t[:, :] for t in ym])
    # ---------------- out_proj (bf16) + exchange + LN1 + FFN + LN2
    with tc.tile_pool(name=f"w3_{l}", bufs=1) as wp3:
        OL = _load_tiles(c, wp3, f"out_lhsT_{l}")
        fT = []
        for mi in range(2):
            t_ = lp.tile([128, N], BF16, name=f"fT{mi}", tag=f"fT{mi}")
            fT.append(t_)
            for n0, nl in NC2:
                ps = pm.tile([128, nl], F32, name="opmm", tag="mm")
                _matsum(c, ps, [OL[k][mi] for k in range(4)], ym, n0, nl)
                nc.scalar.copy(t_[:, n0:n0 + nl], ps[:, :])

        fdram = c.dp.tile([256, N], BF16, name=f"fd{l}", tag="fdram")
        sdram = c.dp.tile([512, N], BF16, name=f"sd{l}", tag="sdram")
        for mi in range(2):
            nc.gpsimd.dma_start(fdram[mi * 128:(mi + 1) * 128, :], fT[mi][:, :])
        nc.gpsimd.collective_compute("AllGather", AL.bypass,
                                     replica_groups=PAIRS,
                                     ins=[fdram.opt()], outs=[sdram.opt()])

        if l == 0:
            _dbg(c, "fT", [t[:, :] for t in fT])
        # ---- collective window fill: trend branch work
        if l == 0:
            _trend_extract(c, c.gp)
        else:
            _trend_mix(c, c.gp)

        # ---- exchange consume
        xnew = []
        for mi in range(2):
            s0 = rp.tile([128, N], BF16, name=f"exs{mi}", tag="exs", bufs=2)
            nc.gpsimd.dma_start(s0[:, :], sdram[mi * 128:(mi + 1) * 128, :])
            s1 = rp.tile([128, N], BF16, name=f"ext{mi}", tag="ext", bufs=2)
            nc.gpsimd.dma_start(s1[:, :],
                                sdram[256 + mi * 128:256 + (mi + 1) * 128, :])
            a_ = rp.tile([128, N], BF16, name=f"exa{mi}", tag="exa", bufs=2)
            nc.gpsimd.tensor_tensor(a_[:, :], s0[:, :], s1[:, :], AL.add)
            # dr = rev(own+partner) - rev(own)  (= partner branch, reversed)
            dr = rp.tile([128, N], BF16, name=f"exd{mi}", tag="exd", bufs=2)
            nc.vector.tensor_tensor(dr[:, :], a_[:, ::-1], fT[mi][:, ::-1],
                                    AL.subtract)
            nc.vector.tensor_tensor(dr[:, :], dr[:, :], fT[mi][:, :], AL.add)
            xv = lp.tile([128, N], BF16, name=f"xnew{mi}", tag=f"xnew{mi}")
            nc.vector.tensor_tensor(xv[:, :], xt[mi][:, :], dr[:, :], AL.add)
            xnew.append(xv)
        if l == 0:
            _dbg(c, "xnew", [t[:, :] for t in xnew])
        n1w = _load(c, lp, f"n1w_{l}")
        n1b = _load(c, lp, f"n1b_{l}")
        xln = _layer_norm(c, rp, xnew, n1w, n1b, lp, f"xln{l}_",
                          pool_mi=(0, 1))

        F1 = _load_tiles(c, wp3, f"f1_lhsT_{l}")
        F2 = _load_tiles(c, wp3, f"f2_lhsT_{l}")
        f1b = _load(c, lp, f"f1b_{l}")
        f2b = _load(c, lp, f"f2b_{l}")
        h1 = []
        for mf in range(2):
            t_ = lp.tile([128, N], BF16, name=f"ffh{mf}", tag=f"xcs{mf}")
            h1.append(t_)
            for n0, nl in NC2:
                ps = pm.tile([128, nl], F32, name="f1mm", tag="mm")
                _matsum(c, ps, [F1[k][mf] for k in range(2)], xln, n0, nl)
                nc.scalar.activation(t_[:, n0:n0 + nl], ps[:, :],
                                     AF.Gelu,
                                     bias=f1b[:, mf:mf + 1])
        xe2 = []
        for mi in range(2):
            xv = lp.tile([128, N], BF16, name=f"xe2{mi}", tag=f"xcs{mi + 2}")
            for n0, nl in NC2:
                ps = pm.tile([128, nl], F32, name="f2mm", tag="mm")
                _matsum(c, ps, [F2[k][mi] for k in range(2)], h1, n0, nl)
                nc.vector.scalar_tensor_tensor(xv[:, n0:n0 + nl], ps[:, :],
                                               f2b[:, mi:mi + 1],
                                               xln[mi][:, n0:n0 + nl],
                                               AL.add, AL.add)
            xe2.append(xv)
        if l == 0:
            _dbg(c, "xe2", [t[:, :] for t in xe2])
        n2w = _load(c, lp, f"n2w_{l}")
        n2b = _load(c, lp, f"n2b_{l}")
        xout = _layer_norm(c, rp, xe2, n2w, n2b, c.gp,
                           "xtB" if l % 2 == 0 else "xtA", pool_mi=(0,))
    return xout


# ---------------------------------------------------------------- entry
def _get_program():
    if "prog" not in _CACHE:
        _CACHE["prog"] = _build()
    return _CACHE["prog"]


def kernel(**inputs):
    nc = _get_program()
    in_maps = [make_core_inputs(inputs, c) for c in range(8)]
    res = run_bass_kernel_spmd(nc, in_maps, list(range(8))).results
    out = np.empty((B, H, N, 1), np.float32)
    for b in range(B):
        out[b, :, :, 0] = res[2 * b]["pred"]
    return out


if __name__ == "__main__":
    print("building program...")
    _get_program()
    print("built ok")
